# revision 10
# baseline (speedup 1.0000x reference)
"""Graphwise KL loss (segment_reduce) on 8 trn2 NeuronCores.

Strategy (v5): bf16 packed f-major inputs with host-precomputed
lq = ln(yp+1e-8); HWDGE (sync-engine) loads 4-deep so the DMA queue
never starves; DVE does pr/d/e1 as bf16 2x tensor-tensor ops; ACT does
lp = Ln(pr+1e-37) and the PSUM->SBUF copies; PE computes 32-element
block sums via col-group matmuls after an 8-matmul HAM warmup.  Each
col-group a uses its own weight matrix W_a (ones at columns 4a..4a+3)
so the four groups' sums land at partitions {36a+b} — distinct mod 16,
spreading the output DMAs over all 16 SDMA engines instead of 4.
Intermediate rings are 3-deep to keep the DVE free of end-of-pipe WAR
stalls; out-DMAs run in two phases (after copy(5), after copy(6)) split
across the gpsimd/sync/scalar queues.

Host (fp64): prefix sums over the block sums + exact f32 partials at
the (<32-element) block prefixes of each segment boundary give
per-segment A_g (e1 sums) and B_g (pr sums); with S_g = max(B_g, EPS):
    total = mean_g (A_g - B_g*ln(S_g)) / S_g
bf16 device noise lands ~1.5e-5 relative (tolerance 2e-2).
"""

import numpy as np

N_TOTAL = 8388608
N_CORES = 8
N_LOCAL = N_TOTAL // N_CORES      # 1048576
P = 128
F_SEQ = (1024, 1024, 1024, 2048, 2048, 512, 512)   # sum = 8192
N_TILES = len(F_SEQ)
F_MAX = max(F_SEQ)
F_OFF = [sum(F_SEQ[:i]) for i in range(N_TILES + 1)]
BLK = 32
GROUPS = 4
N_BUF = 4                         # input buffer depth
N_RING = 3                        # intermediate ring depth
OUT_ROWS = [0, 36, 72, 108]       # stage rows per col-group (spread mod 16)
N_BLOCKS_LOCAL = N_LOCAL // BLK   # 32768
OUT_ELEMS = 2 * N_BLOCKS_LOCAL    # 65536
STAGE_F = OUT_ELEMS // 16         # 4096 f32 per used partition
PHASE_A = 6                       # phase-A out-DMAs after copy(5)
EPS = 1e-8
TINY = 1e-37

_CACHE = {}


def _check_one_wait(nc):
    """Assert no non-EventSemaphore instruction carries more than one wait."""
    bad = []
    for f in nc.m.functions:
        for bb in f.blocks:
            for inst in bb.instructions:
                si = inst.sync_info
                if si and si.on_wait and len(si.on_wait) > 1:
                    if "EventSem" not in type(inst).__name__:
                        bad.append((type(inst).__name__, inst.name, len(si.on_wait)))
    assert not bad, f"multi-wait instructions remain: {bad}"


def _build_program():
    import concourse.bass as bass
    import concourse.mybir as mybir

    f32 = mybir.dt.float32
    bf16 = mybir.dt.bfloat16
    Ln = mybir.ActivationFunctionType.Ln
    Copy = mybir.ActivationFunctionType.Copy

    nc = bass.Bass()

    const_aps = {}
    for val in (TINY, EPS):
        ct = nc.alloc_sbuf_tensor(f"const-f32-{val}", [P, 1], f32)
        nc.const_aps.aps[(f32, val)] = ct.ap()
        const_aps[val] = ct.ap()
    # W_a[p, m] = 1 iff m == 4a + p//32: group a's block sums land at
    # output partitions 32a + 4a + b = 36a + b (distinct mod 16).
    w_blk = [nc.alloc_sbuf_tensor(f"w_blk{a}", [P, 32], bf16) for a in range(GROUPS)]
    t_warm = nc.alloc_sbuf_tensor("t_warm", [P, 512], bf16)
    t_dummy = nc.alloc_sbuf_tensor("t_dummy", [P, 1], f32)

    inp = nc.declare_dram_parameter("inp", [3 * N_LOCAL], bf16, isOutput=False)
    o = nc.declare_dram_parameter("o", [OUT_ELEMS], f32, isOutput=True)

    inp_t = []
    for t in range(N_TILES):
        start = 3 * P * F_OFF[t]
        inp_t.append(
            inp[start : start + 3 * P * F_SEQ[t]].rearrange("(p f) -> p f", p=P)
        )
    o3 = o[:].rearrange("(a m f) -> a m f", a=GROUPS, m=4, f=STAGE_F)

    def bufn(name, shape, dt, n):
        return [nc.alloc_sbuf_tensor(f"{name}{i}", shape, dt).ap() for i in range(n)]

    t_in = bufn("t_in", [P, 3 * F_MAX], bf16, N_BUF)
    t_pr = bufn("t_pr", [P, F_MAX], bf16, N_RING)
    t_lp = bufn("t_lp", [P, F_MAX], bf16, N_RING)
    t_d = bufn("t_d", [P, F_MAX], bf16, N_RING)
    t_e1 = bufn("t_e1", [P, F_MAX], bf16, N_RING)
    stage = nc.alloc_sbuf_tensor("stage", [P, STAGE_F], f32).ap()

    # PSUM: [128, F/2] f32 per tile; tile 6 reuses tile 0's bank.
    ps = [
        nc.alloc_psum_tensor(f"ps{t}", [P, F_SEQ[t] // 2], f32).ap()
        for t in range(N_TILES - 1)
    ]
    ps.append(ps[0][:, 0 : F_SEQ[N_TILES - 1] // 2])

    s_in = [nc.alloc_semaphore(f"s_in{i}") for i in range(N_BUF)]  # +16 per load
    s_init = nc.alloc_semaphore("s_init")
    s_act = nc.alloc_semaphore("s_act")  # +1 per lp  (-> t+1)
    s_cp = nc.alloc_semaphore("s_cp")    # +1 per psum copy (-> t+1)
    s_dve = nc.alloc_semaphore("s_dve")  # +1 per DVE op
    s_pe = nc.alloc_semaphore("s_pe")    # +1 per tile's matmul group (-> t+1)
    s_out = [nc.alloc_semaphore(f"s_out{g}") for g in range(GROUPS)]

    # DVE op order: pr0, pr1, then per tile [d(t), e1(t), pr(t+2)].
    dve_idx = {}
    n = 0
    order = [("pr", 0), ("pr", 1)]
    for t in range(N_TILES):
        order.append(("d", t))
        order.append(("e1", t))
        if t + 2 < N_TILES:
            order.append(("pr", t + 2))
    for kind, t in order:
        n += 1
        dve_idx[(kind, t)] = n

    def views(t):
        buf, F = t % N_RING, F_SEQ[t]
        ib = t % N_BUF
        return (
            t_in[ib][:, 0:F],                 # lq
            t_in[ib][:, F : 2 * F],           # yt
            t_in[ib][:, 2 * F : 3 * F],       # w
            t_pr[buf][:, 0:F],
            t_lp[buf][:, 0:F],
            t_d[buf][:, 0:F],
            t_e1[buf][:, 0:F],
        )

    def in_count(t):
        return 16 * (t // N_BUF + 1)

    ca, cb = F_OFF[PHASE_A] // 2, STAGE_F

    with nc.Block() as block:

        @block.sync
        def _(sy):
            for t in range(N_TILES):
                mm = sy.dma_start(t_in[t % N_BUF][:, 0 : 3 * F_SEQ[t]], inp_t[t])
                mm.then_inc(s_in[t % N_BUF], 16)
                if t >= N_BUF:
                    # t_in[buf]'s last reader is d(t - N_BUF)
                    mm._wait_ge(s_dve, dve_idx[("d", t - N_BUF)])
            # phase-A out-DMAs, groups 2,3
            sy.dma_start(
                o3[2][:, 0:ca], stage[OUT_ROWS[2] : OUT_ROWS[2] + 4, 0:ca]
            ).then_inc(s_out[2], 16)._wait_ge(s_cp, PHASE_A)
            sy.dma_start(
                o3[3][:, 0:ca], stage[OUT_ROWS[3] : OUT_ROWS[3] + 4, 0:ca]
            ).then_inc(s_out[3], 16)

        @block.gpsimd
        def _(g):
            for val in (TINY, EPS):
                g.memset(const_aps[val], val)
            for a in range(GROUPS):
                g.memset(w_blk[a].ap(), 0.0)
                for b in range(4):
                    g.memset(
                        w_blk[a].ap()[32 * b : 32 * b + 32, 4 * a + b : 4 * a + b + 1],
                        1.0,
                    )
            g.memset(t_warm.ap(), 0.0).then_inc(s_init, 1)
            # phase-A out-DMAs, groups 0,1
            g.dma_start(
                o3[0][:, 0:ca], stage[OUT_ROWS[0] : OUT_ROWS[0] + 4, 0:ca]
            ).then_inc(s_out[0], 16)._wait_ge(s_cp, PHASE_A)
            g.dma_start(
                o3[1][:, 0:ca], stage[OUT_ROWS[1] : OUT_ROWS[1] + 4, 0:ca]
            ).then_inc(s_out[1], 16)
            # phase-B, groups 0,1
            g.dma_start(
                o3[0][:, ca:cb], stage[OUT_ROWS[0] : OUT_ROWS[0] + 4, ca:cb]
            ).then_inc(s_out[0], 16)._wait_ge(s_cp, N_TILES)
            g.dma_start(
                o3[1][:, ca:cb], stage[OUT_ROWS[1] : OUT_ROWS[1] + 4, ca:cb]
            ).then_inc(s_out[1], 16)
            for gi in range(GROUPS):
                g.wait_ge(s_out[gi], 32)

        @block.scalar
        def _(s):
            # Warm the Ln table set while the first tiles load.
            s.activation(t_dummy.ap(), const_aps[TINY], Ln, bias=EPS)._wait_ge(
                s_init, 1
            )
            for t in range(N_TILES):
                _lq, _yt, _w, pr, lp, _d, _e1 = views(t)
                # lp[buf] WAR vs d(t-3) covered: idx(pr(t)) > idx(d(t-3))
                s.activation(lp, pr, Ln, bias=TINY).then_inc(s_act, 1)._wait_ge(
                    s_dve, dve_idx[("pr", t)]
                )
                if t >= 1:
                    tt = t - 1
                    s.activation(
                        stage[:, F_OFF[tt] // 2 : F_OFF[tt + 1] // 2], ps[tt], Copy
                    ).then_inc(s_cp, 1)._wait_ge(s_pe, tt + 1)
            tt = N_TILES - 1
            s.activation(
                stage[:, F_OFF[tt] // 2 : F_OFF[tt + 1] // 2], ps[tt], Copy
            ).then_inc(s_cp, 1)._wait_ge(s_pe, tt + 1)
            # phase-B out-DMAs, groups 2,3 (ACT's empty HWDGE ring)
            s.dma_start(
                o3[2][:, ca:cb], stage[OUT_ROWS[2] : OUT_ROWS[2] + 4, ca:cb]
            ).then_inc(s_out[2], 16)
            s.dma_start(
                o3[3][:, ca:cb], stage[OUT_ROWS[3] : OUT_ROWS[3] + 4, ca:cb]
            ).then_inc(s_out[3], 16)

        @block.vector
        def _(v):
            for kind, t in order:
                lq, yt, w, pr, lp, d, e1 = views(t)
                if kind == "pr":
                    if t >= N_RING:
                        # PE(t-3) done => e1/d/lp(t-3) done: covers the
                        # pr/e1[ring] WAR hazards transitively
                        v.wait_ge(s_pe, t - 2)
                    v.tensor_mul(pr, yt, w).then_inc(s_dve, 1)._wait_ge(
                        s_in[t % N_BUF], in_count(t)
                    )
                elif kind == "d":
                    v.tensor_sub(d, lp, lq).then_inc(s_dve, 1)._wait_ge(s_act, t + 1)
                else:  # e1: same-engine RAW needs an explicit wait
                    v.tensor_mul(e1, pr, d).then_inc(s_dve, 1)._wait_ge(
                        s_dve, dve_idx[("d", t)]
                    )

        @block.tensor
        def _(te):
            # 8 back-to-back matmuls flip the PE HAM to 8/8 before the
            # first real matmul group arrives.
            for i in range(8):
                mm = te.matmul(
                    ps[4][0:32, 0:512],
                    w_blk[0].ap(),
                    t_warm.ap(),
                    start=True,
                    stop=True,
                )
                if i == 0:
                    mm._wait_ge(s_init, 1)
            for t in range(N_TILES):
                _lq, _yt, _w, pr, _lp, _d, e1 = views(t)
                npg = F_SEQ[t] // GROUPS
                if t == N_TILES - 1:
                    # ps[6] is a view of ps[0]: copy(0) must be done
                    te.wait_ge(s_cp, 1)
                for a in range(GROUPS):
                    mm = te.matmul(
                        ps[t][32 * a : 32 * a + 32, npg : 2 * npg],
                        w_blk[a].ap(),
                        pr[:, npg * a : npg * (a + 1)],
                        start=True,
                        stop=True,
                        tile_position=(0, 32 * a),
                    )
                    if a == 0:
                        mm._wait_ge(s_dve, dve_idx[("pr", t)])
                for a in range(GROUPS):
                    mm = te.matmul(
                        ps[t][32 * a : 32 * a + 32, 0:npg],
                        w_blk[a].ap(),
                        e1[:, npg * a : npg * (a + 1)],
                        start=True,
                        stop=True,
                        tile_position=(0, 32 * a),
                    )
                    if a == 0:
                        mm._wait_ge(s_dve, dve_idx[("e1", t)])
                mm.then_inc(s_pe, 1)  # matmuls complete in pc order

    _check_one_wait(nc)
    return nc


def _get_program():
    if "nc" not in _CACHE:
        _CACHE["nc"] = _build_program()
    return _CACHE["nc"]


def _pack_inputs(yp, yt, w):
    """bf16 f-major packed input per core: per tile, row p = [lq | yt | w]."""
    import ml_dtypes

    bf16 = ml_dtypes.bfloat16
    lq = np.log(yp + np.float32(EPS))
    packed = np.empty((N_CORES, 3 * N_LOCAL), dtype=bf16)
    for t in range(N_TILES):
        F = F_SEQ[t]
        lo, hi = F_OFF[t] * P, F_OFF[t + 1] * P
        dst = packed[:, 3 * lo : 3 * hi].reshape(N_CORES, P, 3 * F)
        for k, arr in enumerate((lq, yt, w)):
            src = arr.reshape(N_CORES, N_LOCAL)[:, lo:hi]
            dst[:, :, k * F : (k + 1) * F] = src.reshape(
                N_CORES, F, P
            ).transpose(0, 2, 1)
    return packed


def _run_device(yp, yt, w, trace=False):
    from concourse.bass_utils import run_bass_kernel_spmd

    nc = _get_program()
    packed = _pack_inputs(yp, yt, w)
    in_maps = [{"inp": packed[k]} for k in range(N_CORES)]
    res = run_bass_kernel_spmd(nc, in_maps, list(range(N_CORES)), trace=trace)
    bs1_parts, bs2_parts = [], []
    for r in res.results:
        ob = r["o"].reshape(GROUPS, 4, STAGE_F)
        b1 = np.empty(N_BLOCKS_LOCAL, np.float32)
        b2 = np.empty(N_BLOCKS_LOCAL, np.float32)
        for t in range(N_TILES):
            F = F_SEQ[t]
            npg = F // GROUPS
            blk_lo = F_OFF[t] * P // BLK
            nblk = F * P // BLK
            for c, bx in ((0, b1), (1, b2)):
                chunk = ob[:, :, F_OFF[t] // 2 + c * npg : F_OFF[t] // 2 + (c + 1) * npg]
                # chunk[a, m, n] -> block (a*npg + n)*4 + m: order (a, n, m)
                bx[blk_lo : blk_lo + nblk] = chunk.transpose(0, 2, 1).reshape(-1)
        bs1_parts.append(b1)
        bs2_parts.append(b2)
    return np.concatenate(bs1_parts), np.concatenate(bs2_parts), res


def kernel(y_pred, y_true, weight, segment_ptr, _trace=False):
    yp = np.ascontiguousarray(np.asarray(y_pred), dtype=np.float32).reshape(-1)
    yt = np.ascontiguousarray(np.asarray(y_true), dtype=np.float32).reshape(-1)
    w = np.ascontiguousarray(np.asarray(weight), dtype=np.float32).reshape(-1)
    ptr = np.asarray(segment_ptr).astype(np.int64).reshape(-1)
    n = yp.shape[0]
    G = ptr.shape[0] - 1
    assert n == N_TOTAL, f"kernel compiled for N={N_TOTAL}, got {n}"

    bs1, bs2, res = _run_device(yp, yt, w, trace=_trace)
    _CACHE["last_res"] = res

    # ---- host assembly in fp64 ----
    pre1 = np.empty(bs1.shape[0] + 1)
    pre1[0] = 0.0
    np.cumsum(bs1, dtype=np.float64, out=pre1[1:])
    pre2 = np.empty(bs2.shape[0] + 1)
    pre2[0] = 0.0
    np.cumsum(bs2, dtype=np.float64, out=pre2[1:])

    ptrc = np.clip(ptr, 0, n)
    b_idx = ptrc // BLK
    r = ptrc - b_idx * BLK
    seg_off = np.concatenate([[0], np.cumsum(r)])
    tot = int(seg_off[-1])
    part1 = np.zeros(ptrc.shape[0])
    part2 = np.zeros(ptrc.shape[0])
    if tot > 0:
        idx = np.repeat(ptrc - r, r) + (np.arange(tot) - np.repeat(seg_off[:-1], r))
        pr_h = yt[idx].astype(np.float64) * w[idx].astype(np.float64)
        e1_h = pr_h * (np.log(pr_h + TINY) - np.log(yp[idx].astype(np.float64) + EPS))
        nz = r > 0
        red_idx = np.minimum(seg_off[:-1][nz], tot - 1).astype(np.int64)
        part1[nz] = np.add.reduceat(e1_h, red_idx)
        part2[nz] = np.add.reduceat(pr_h, red_idx)

    C1 = pre1[b_idx] + part1
    C2 = pre2[b_idx] + part2
    A = np.diff(C1)
    Bg = np.diff(C2)
    S = np.maximum(Bg, EPS)
    total = np.sum((A - Bg * np.log(S)) / S) / max(G, 1)
    return np.float32(total)


# revision 13
# speedup vs baseline: 1.0360x; 1.0360x over previous
"""Graphwise KL loss (segment_reduce) on 8 trn2 NeuronCores.

Strategy (v5): bf16 packed f-major inputs with host-precomputed
lq = ln(yp+1e-8); HWDGE (sync-engine) loads 4-deep so the DMA queue
never starves; DVE does pr/d/e1 as bf16 2x tensor-tensor ops; ACT does
lp = Ln(pr+1e-37) and the PSUM->SBUF copies; PE computes 32-element
block sums via col-group matmuls after an 8-matmul HAM warmup.  Each
col-group a uses its own weight matrix W_a (ones at columns 4a..4a+3)
so the four groups' sums land at partitions {36a+b} — distinct mod 16,
spreading the output DMAs over all 16 SDMA engines instead of 4.
Intermediate rings are 3-deep to keep the DVE free of end-of-pipe WAR
stalls; out-DMAs run in two phases (after copy(5), after copy(6)) split
across the gpsimd/sync/scalar queues.

Host (fp64): prefix sums over the block sums + exact f32 partials at
the (<32-element) block prefixes of each segment boundary give
per-segment A_g (e1 sums) and B_g (pr sums); with S_g = max(B_g, EPS):
    total = mean_g (A_g - B_g*ln(S_g)) / S_g
bf16 device noise lands ~1.5e-5 relative (tolerance 2e-2).
"""

import numpy as np

N_TOTAL = 8388608
N_CORES = 8
N_LOCAL = N_TOTAL // N_CORES      # 1048576
P = 128
F_SEQ = (1024, 1024, 1024, 2048, 2048, 512, 512)   # sum = 8192
N_TILES = len(F_SEQ)
F_MAX = max(F_SEQ)
F_OFF = [sum(F_SEQ[:i]) for i in range(N_TILES + 1)]
BLK = 32
GROUPS = 4
N_BUF = 4                         # input buffer depth
N_RING = 3                        # intermediate ring depth
OUT_ROWS = [0, 36, 72, 108]       # stage rows per col-group (spread mod 16)
N_BLOCKS_LOCAL = N_LOCAL // BLK   # 32768
OUT_ELEMS = 2 * N_BLOCKS_LOCAL    # 65536
STAGE_F = OUT_ELEMS // 16         # 4096 f32 per used partition
PHASE_A = 6                       # phase-A out-DMAs after copy(5)
EPS = 1e-8
TINY = 1e-37

_CACHE = {}


def _check_one_wait(nc):
    """Assert no non-EventSemaphore instruction carries more than one wait."""
    bad = []
    for f in nc.m.functions:
        for bb in f.blocks:
            for inst in bb.instructions:
                si = inst.sync_info
                if si and si.on_wait and len(si.on_wait) > 1:
                    if "EventSem" not in type(inst).__name__:
                        bad.append((type(inst).__name__, inst.name, len(si.on_wait)))
    assert not bad, f"multi-wait instructions remain: {bad}"


def _build_program():
    import concourse.bass as bass
    import concourse.mybir as mybir

    f32 = mybir.dt.float32
    bf16 = mybir.dt.bfloat16
    Ln = mybir.ActivationFunctionType.Ln
    Copy = mybir.ActivationFunctionType.Copy

    nc = bass.Bass()

    const_aps = {}
    for val in (TINY, EPS):
        ct = nc.alloc_sbuf_tensor(f"const-f32-{val}", [P, 1], f32)
        nc.const_aps.aps[(f32, val)] = ct.ap()
        const_aps[val] = ct.ap()
    # W_a[p, m] = 1 iff m == 4a + p//32: group a's block sums land at
    # output partitions 32a + 4a + b = 36a + b (distinct mod 16).
    w_blk = [nc.alloc_sbuf_tensor(f"w_blk{a}", [P, 32], bf16) for a in range(GROUPS)]
    t_warm = nc.alloc_sbuf_tensor("t_warm", [P, 512], bf16)
    t_dummy = nc.alloc_sbuf_tensor("t_dummy", [P, 1], f32)

    inp = nc.declare_dram_parameter("inp", [3 * N_LOCAL], bf16, isOutput=False)
    o = nc.declare_dram_parameter("o", [OUT_ELEMS], f32, isOutput=True)

    inp_t = []
    for t in range(N_TILES):
        start = 3 * P * F_OFF[t]
        inp_t.append(
            inp[start : start + 3 * P * F_SEQ[t]].rearrange("(p f) -> p f", p=P)
        )
    o3 = o[:].rearrange("(a m f) -> a m f", a=GROUPS, m=4, f=STAGE_F)

    def bufn(name, shape, dt, n):
        return [nc.alloc_sbuf_tensor(f"{name}{i}", shape, dt).ap() for i in range(n)]

    t_in = bufn("t_in", [P, 3 * F_MAX], bf16, N_BUF)
    t_pr = bufn("t_pr", [P, F_MAX], bf16, N_RING)
    t_lp = bufn("t_lp", [P, F_MAX], bf16, N_RING)
    t_d = bufn("t_d", [P, F_MAX], bf16, N_RING)
    t_e1 = bufn("t_e1", [P, F_MAX], bf16, N_RING)
    stage = nc.alloc_sbuf_tensor("stage", [P, STAGE_F], f32).ap()

    # PSUM: [128, F/2] f32 per tile; tile 6 reuses tile 0's bank.
    ps = [
        nc.alloc_psum_tensor(f"ps{t}", [P, F_SEQ[t] // 2], f32).ap()
        for t in range(N_TILES - 1)
    ]
    ps.append(ps[0][:, 0 : F_SEQ[N_TILES - 1] // 2])

    s_in = [nc.alloc_semaphore(f"s_in{i}") for i in range(N_BUF)]  # +16 per load
    s_init = nc.alloc_semaphore("s_init")
    s_act = nc.alloc_semaphore("s_act")  # +1 per lp  (-> t+1)
    s_cp = nc.alloc_semaphore("s_cp")    # +1 per psum copy (-> t+1)
    s_dve = nc.alloc_semaphore("s_dve")  # +1 per DVE op
    s_pe = nc.alloc_semaphore("s_pe")    # +1 per tile's matmul group (-> t+1)
    s_out = [nc.alloc_semaphore(f"s_out{g}") for g in range(GROUPS)]

    # DVE op order: pr0, pr1, then per tile [d(t), e1(t), pr(t+2)].
    dve_idx = {}
    n = 0
    order = [("pr", 0), ("pr", 1)]
    for t in range(N_TILES):
        order.append(("d", t))
        order.append(("e1", t))
        if t + 2 < N_TILES:
            order.append(("pr", t + 2))
    for kind, t in order:
        n += 1
        dve_idx[(kind, t)] = n

    def views(t):
        buf, F = t % N_RING, F_SEQ[t]
        ib = t % N_BUF
        return (
            t_in[ib][:, 0:F],                 # lq
            t_in[ib][:, F : 2 * F],           # yt
            t_in[ib][:, 2 * F : 3 * F],       # w
            t_pr[buf][:, 0:F],
            t_lp[buf][:, 0:F],
            t_d[buf][:, 0:F],
            t_e1[buf][:, 0:F],
        )

    def in_count(t):
        return 16 * (t // N_BUF + 1)

    ca, cb = F_OFF[PHASE_A] // 2, STAGE_F

    with nc.Block() as block:

        def emit_load(eng, t):
            mm = eng.dma_start(t_in[t % N_BUF][:, 0 : 3 * F_SEQ[t]], inp_t[t])
            mm.then_inc(s_in[t % N_BUF], 16)
            if t >= N_BUF:
                # t_in[buf]'s last reader is d(t - N_BUF)
                mm._wait_ge(s_dve, dve_idx[("d", t - N_BUF)])

        @block.sync
        def _(sy):
            # even tiles on the SP HWDGE ring; odd tiles ride the ACT ring
            # (one ring alone sustains only ~210 GB/s; two interleave to
            # full HBM rate)
            for t in range(0, N_TILES, 2):
                emit_load(sy, t)
            # phase-A out-DMAs, groups 2,3
            sy.dma_start(
                o3[2][:, 0:ca], stage[OUT_ROWS[2] : OUT_ROWS[2] + 4, 0:ca]
            ).then_inc(s_out[2], 16)._wait_ge(s_cp, PHASE_A)
            sy.dma_start(
                o3[3][:, 0:ca], stage[OUT_ROWS[3] : OUT_ROWS[3] + 4, 0:ca]
            ).then_inc(s_out[3], 16)

        @block.gpsimd
        def _(g):
            for val in (TINY, EPS):
                g.memset(const_aps[val], val)
            for a in range(GROUPS):
                g.memset(w_blk[a].ap(), 0.0)
                for b in range(4):
                    g.memset(
                        w_blk[a].ap()[32 * b : 32 * b + 32, 4 * a + b : 4 * a + b + 1],
                        1.0,
                    )
            g.memset(t_warm.ap(), 0.0).then_inc(s_init, 1)
            # phase-A out-DMAs, groups 0,1
            g.dma_start(
                o3[0][:, 0:ca], stage[OUT_ROWS[0] : OUT_ROWS[0] + 4, 0:ca]
            ).then_inc(s_out[0], 16)._wait_ge(s_cp, PHASE_A)
            g.dma_start(
                o3[1][:, 0:ca], stage[OUT_ROWS[1] : OUT_ROWS[1] + 4, 0:ca]
            ).then_inc(s_out[1], 16)
            # phase-B, groups 0,1
            g.dma_start(
                o3[0][:, ca:cb], stage[OUT_ROWS[0] : OUT_ROWS[0] + 4, ca:cb]
            ).then_inc(s_out[0], 16)._wait_ge(s_cp, N_TILES)
            g.dma_start(
                o3[1][:, ca:cb], stage[OUT_ROWS[1] : OUT_ROWS[1] + 4, ca:cb]
            ).then_inc(s_out[1], 16)
            for gi in range(GROUPS):
                g.wait_ge(s_out[gi], 32)

        @block.scalar
        def _(s):
            # Ungated odd-tile loads go first so the ACT ring starts
            # streaming immediately; load(t)'s gate d(t-4) is implied by
            # lp(t-3)'s own gate, so in-stream loads never stall ACT.
            for t in range(1, min(N_BUF, N_TILES), 2):
                emit_load(s, t)
            # Warm the Ln table set while the first tiles load.
            s.activation(t_dummy.ap(), const_aps[TINY], Ln, bias=EPS)._wait_ge(
                s_init, 1
            )
            for t in range(N_TILES):
                _lq, _yt, _w, pr, lp, _d, _e1 = views(t)
                # lp[buf] WAR vs d(t-3) covered: idx(pr(t)) > idx(d(t-3))
                s.activation(lp, pr, Ln, bias=TINY).then_inc(s_act, 1)._wait_ge(
                    s_dve, dve_idx[("pr", t)]
                )
                tl = t + N_BUF
                if N_BUF <= tl < N_TILES and tl % 2 == 1:
                    # gate d(tl-4) = d(t) finishes before lp(t) does
                    emit_load(s, tl)
                if t >= 1:
                    tt = t - 1
                    s.activation(
                        stage[:, F_OFF[tt] // 2 : F_OFF[tt + 1] // 2], ps[tt], Copy
                    ).then_inc(s_cp, 1)._wait_ge(s_pe, tt + 1)
            tt = N_TILES - 1
            s.activation(
                stage[:, F_OFF[tt] // 2 : F_OFF[tt + 1] // 2], ps[tt], Copy
            ).then_inc(s_cp, 1)._wait_ge(s_pe, tt + 1)
            # phase-B out-DMAs, groups 2,3 (ACT's empty HWDGE ring)
            s.dma_start(
                o3[2][:, ca:cb], stage[OUT_ROWS[2] : OUT_ROWS[2] + 4, ca:cb]
            ).then_inc(s_out[2], 16)
            s.dma_start(
                o3[3][:, ca:cb], stage[OUT_ROWS[3] : OUT_ROWS[3] + 4, ca:cb]
            ).then_inc(s_out[3], 16)

        @block.vector
        def _(v):
            for kind, t in order:
                lq, yt, w, pr, lp, d, e1 = views(t)
                if kind == "pr":
                    if t >= N_RING:
                        # PE(t-3) done => e1/d/lp(t-3) done: covers the
                        # pr/e1[ring] WAR hazards transitively
                        v.wait_ge(s_pe, t - 2)
                    v.tensor_mul(pr, yt, w).then_inc(s_dve, 1)._wait_ge(
                        s_in[t % N_BUF], in_count(t)
                    )
                elif kind == "d":
                    v.tensor_sub(d, lp, lq).then_inc(s_dve, 1)._wait_ge(s_act, t + 1)
                else:  # e1: same-engine RAW needs an explicit wait
                    v.tensor_mul(e1, pr, d).then_inc(s_dve, 1)._wait_ge(
                        s_dve, dve_idx[("d", t)]
                    )

        @block.tensor
        def _(te):
            # 8 back-to-back matmuls flip the PE HAM to 8/8 before the
            # first real matmul group arrives.
            for i in range(8):
                mm = te.matmul(
                    ps[4][0:32, 0:512],
                    w_blk[0].ap(),
                    t_warm.ap(),
                    start=True,
                    stop=True,
                )
                if i == 0:
                    mm._wait_ge(s_init, 1)
            for t in range(N_TILES):
                _lq, _yt, _w, pr, _lp, _d, e1 = views(t)
                npg = F_SEQ[t] // GROUPS
                if t == N_TILES - 1:
                    # ps[6] is a view of ps[0]: copy(0) must be done
                    te.wait_ge(s_cp, 1)
                for a in range(GROUPS):
                    mm = te.matmul(
                        ps[t][32 * a : 32 * a + 32, npg : 2 * npg],
                        w_blk[a].ap(),
                        pr[:, npg * a : npg * (a + 1)],
                        start=True,
                        stop=True,
                        tile_position=(0, 32 * a),
                    )
                    if a == 0:
                        mm._wait_ge(s_dve, dve_idx[("pr", t)])
                for a in range(GROUPS):
                    mm = te.matmul(
                        ps[t][32 * a : 32 * a + 32, 0:npg],
                        w_blk[a].ap(),
                        e1[:, npg * a : npg * (a + 1)],
                        start=True,
                        stop=True,
                        tile_position=(0, 32 * a),
                    )
                    if a == 0:
                        mm._wait_ge(s_dve, dve_idx[("e1", t)])
                mm.then_inc(s_pe, 1)  # matmuls complete in pc order

    _check_one_wait(nc)
    return nc


def _get_program():
    if "nc" not in _CACHE:
        _CACHE["nc"] = _build_program()
    return _CACHE["nc"]


def _pack_inputs(yp, yt, w):
    """bf16 f-major packed input per core: per tile, row p = [lq | yt | w]."""
    import ml_dtypes

    bf16 = ml_dtypes.bfloat16
    lq = np.log(yp + np.float32(EPS))
    packed = np.empty((N_CORES, 3 * N_LOCAL), dtype=bf16)
    for t in range(N_TILES):
        F = F_SEQ[t]
        lo, hi = F_OFF[t] * P, F_OFF[t + 1] * P
        dst = packed[:, 3 * lo : 3 * hi].reshape(N_CORES, P, 3 * F)
        for k, arr in enumerate((lq, yt, w)):
            src = arr.reshape(N_CORES, N_LOCAL)[:, lo:hi]
            dst[:, :, k * F : (k + 1) * F] = src.reshape(
                N_CORES, F, P
            ).transpose(0, 2, 1)
    return packed


def _run_device(yp, yt, w, trace=False):
    from concourse.bass_utils import run_bass_kernel_spmd

    nc = _get_program()
    packed = _pack_inputs(yp, yt, w)
    in_maps = [{"inp": packed[k]} for k in range(N_CORES)]
    res = run_bass_kernel_spmd(nc, in_maps, list(range(N_CORES)), trace=trace)
    bs1_parts, bs2_parts = [], []
    for r in res.results:
        ob = r["o"].reshape(GROUPS, 4, STAGE_F)
        b1 = np.empty(N_BLOCKS_LOCAL, np.float32)
        b2 = np.empty(N_BLOCKS_LOCAL, np.float32)
        for t in range(N_TILES):
            F = F_SEQ[t]
            npg = F // GROUPS
            blk_lo = F_OFF[t] * P // BLK
            nblk = F * P // BLK
            for c, bx in ((0, b1), (1, b2)):
                chunk = ob[:, :, F_OFF[t] // 2 + c * npg : F_OFF[t] // 2 + (c + 1) * npg]
                # chunk[a, m, n] -> block (a*npg + n)*4 + m: order (a, n, m)
                bx[blk_lo : blk_lo + nblk] = chunk.transpose(0, 2, 1).reshape(-1)
        bs1_parts.append(b1)
        bs2_parts.append(b2)
    return np.concatenate(bs1_parts), np.concatenate(bs2_parts), res


def kernel(y_pred, y_true, weight, segment_ptr, _trace=False):
    yp = np.ascontiguousarray(np.asarray(y_pred), dtype=np.float32).reshape(-1)
    yt = np.ascontiguousarray(np.asarray(y_true), dtype=np.float32).reshape(-1)
    w = np.ascontiguousarray(np.asarray(weight), dtype=np.float32).reshape(-1)
    ptr = np.asarray(segment_ptr).astype(np.int64).reshape(-1)
    n = yp.shape[0]
    G = ptr.shape[0] - 1
    assert n == N_TOTAL, f"kernel compiled for N={N_TOTAL}, got {n}"

    bs1, bs2, res = _run_device(yp, yt, w, trace=_trace)
    _CACHE["last_res"] = res

    # ---- host assembly in fp64 ----
    pre1 = np.empty(bs1.shape[0] + 1)
    pre1[0] = 0.0
    np.cumsum(bs1, dtype=np.float64, out=pre1[1:])
    pre2 = np.empty(bs2.shape[0] + 1)
    pre2[0] = 0.0
    np.cumsum(bs2, dtype=np.float64, out=pre2[1:])

    ptrc = np.clip(ptr, 0, n)
    b_idx = ptrc // BLK
    r = ptrc - b_idx * BLK
    seg_off = np.concatenate([[0], np.cumsum(r)])
    tot = int(seg_off[-1])
    part1 = np.zeros(ptrc.shape[0])
    part2 = np.zeros(ptrc.shape[0])
    if tot > 0:
        idx = np.repeat(ptrc - r, r) + (np.arange(tot) - np.repeat(seg_off[:-1], r))
        pr_h = yt[idx].astype(np.float64) * w[idx].astype(np.float64)
        e1_h = pr_h * (np.log(pr_h + TINY) - np.log(yp[idx].astype(np.float64) + EPS))
        nz = r > 0
        red_idx = np.minimum(seg_off[:-1][nz], tot - 1).astype(np.int64)
        part1[nz] = np.add.reduceat(e1_h, red_idx)
        part2[nz] = np.add.reduceat(pr_h, red_idx)

    C1 = pre1[b_idx] + part1
    C2 = pre2[b_idx] + part2
    A = np.diff(C1)
    Bg = np.diff(C2)
    S = np.maximum(Bg, EPS)
    total = np.sum((A - Bg * np.log(S)) / S) / max(G, 1)
    return np.float32(total)
